# revision 1
# baseline (speedup 1.0000x reference)
"""Trainium2 Bass kernel for nn_AttentionBlock (S=4096, H=1024, NH=2, DS=64).

Strategy: sequence parallelism over queries (512 per core on 8 cores).
K/V projections are replicated on every core (cheaper than collectives here).
All matmuls run in float32r (full PE rate, ~1e-4 relative rounding).

Pipeline per core:
  1. PE-transpose x (8 key-blocks, double-buffered) -> K^T, V streamed to DRAM
     scratch; Q^T for own query slice (1/sqrt(hd) folded into eviction).
  2. Per head: S^T chunks -> fused exp+mask eviction (ACT) -> softmax
     denominators via ones-matmul; ctx^T accumulated over 32 key chunks;
     divided by denominators at eviction.
  3. Out-projection directly in natural [q, H] layout (ctx^T as stationary),
     residual fused into PSUM eviction, LayerNorm via bn_stats.
"""

import math
import sys

sys.path.insert(0, "/opt/trn_rl_repo")

import numpy as np

import concourse.bass as bass
import concourse.mybir as mybir
import concourse.tile as tile
from concourse import bacc
from concourse.bass_utils import run_bass_kernel_spmd

S, H, NH, DS = 4096, 1024, 2, 64
HD = H // NH            # 512
NC = 8                  # cores
SQ = S // NC            # 512 queries per core
EPS = 1e-5
F32 = mybir.dt.float32
F32R = mybir.dt.float32r
AF = mybir.ActivationFunctionType
ALU = mybir.AluOpType

KC = S // 128           # 32 key chunks of 128
HC = H // 128           # 8 hidden chunks of 128
QB = SQ // 128          # 4 query chunks of 128


def build_program(debug=False):
    nc = bacc.Bacc("TRN2", target_bir_lowering=False, debug=False, num_devices=NC)

    # ---- DRAM I/O ----
    x = nc.dram_tensor("x", [S, H], F32, kind="ExternalInput")
    xq = nc.dram_tensor("xq", [SQ, H], F32, kind="ExternalInput")
    wqT = nc.dram_tensor("wqT", [H, H], F32R, kind="ExternalInput")
    wkT = nc.dram_tensor("wkT", [H, H], F32R, kind="ExternalInput")
    wvT = nc.dram_tensor("wvT", [H, H], F32R, kind="ExternalInput")
    woT = nc.dram_tensor("woT", [H, H], F32R, kind="ExternalInput")
    wsT = nc.dram_tensor("wsT", [DS, H], F32R, kind="ExternalInput")
    sdat = nc.dram_tensor("sdat", [DS, 1], F32R, kind="ExternalInput")
    bsv = nc.dram_tensor("bsv", [H], F32, kind="ExternalInput")
    mbias = nc.dram_tensor("mbias", [128, KC], F32, kind="ExternalInput")
    onescol = nc.dram_tensor("onescol", [128, 1], F32R, kind="ExternalInput")
    onesrow = nc.dram_tensor("onesrow", [1, 128], F32R, kind="ExternalInput")
    identd = nc.dram_tensor("identd", [128, 128], F32R, kind="ExternalInput")
    lnw = nc.dram_tensor("lnw", [H], F32, kind="ExternalInput")
    lnb = nc.dram_tensor("lnb", [H], F32, kind="ExternalInput")
    out = nc.dram_tensor("out", [SQ, H], F32, kind="ExternalOutput")
    if debug:
        dsemb = nc.dram_tensor("dsemb", [128, HC], F32, kind="ExternalOutput")
        dkbias = nc.dram_tensor("dkbias", [128, HC], F32, kind="ExternalOutput")
        dvb = nc.dram_tensor("dvb", [1, H], F32, kind="ExternalOutput")
        dxT = nc.dram_tensor("dxT", [128, 512], F32, kind="ExternalOutput")
        dqT = nc.dram_tensor("dqT", [128, 512], F32, kind="ExternalOutput")
        dPT = nc.dram_tensor("dPT", [128, 512], F32, kind="ExternalOutput")
        dl = nc.dram_tensor("dl", [NH, SQ], F32, kind="ExternalOutput")
        dctx = nc.dram_tensor("dctx", [128, 512], F32, kind="ExternalOutput")
        doutT = nc.dram_tensor("doutT", [128, 512], F32, kind="ExternalOutput")
        dKT = nc.dram_tensor("dKT", [128, 4, 512], F32, kind="ExternalOutput")
        dST = nc.dram_tensor("dST", [128, 512], F32, kind="ExternalOutput")

    inv_sqrt_hd = 1.0 / math.sqrt(HD)

    with tile.TileContext(nc) as tc:
        with (
            tc.tile_pool(name="consts", bufs=1) as consts,
            tc.tile_pool(name="persist", bufs=1) as persist,
            tc.tile_pool(name="kvin", bufs=2) as kvin,
            tc.tile_pool(name="rlp", bufs=1) as rlp,
            tc.tile_pool(name="dram", bufs=1, space="DRAM") as dram,
        ):
            # ---- packed constants: f32r slot + f32 slot (verifier needs
            # tensor-uniform fp32r rounding, so keep dtypes per tile) ----
            Ar = consts.tile([128, 132], F32R)   # 0:128 ident | 128 ones | 129 sd
            ident = Ar[:, 0:128]
            nc.sync.dma_start(ident, identd[:, :])
            ones_sb = Ar[:, 128:129]
            nc.sync.dma_start(ones_sb, onescol[:, :])
            sd_sb = Ar[0:64, 129:130]
            nc.sync.dma_start(sd_sb, sdat[:, :])
            Af = consts.tile([128, 36], F32)     # 0:32 maskbias | 32 zero | 33 eps
            mb_sb = Af[:, 0:32]
            nc.sync.dma_start(mb_sb, mbias[:, :])
            zb_sb = Af[:, 32:33]
            nc.vector.memset(zb_sb, 0.0)
            eps_sb = Af[:, 33:34]
            nc.vector.memset(eps_sb, EPS)
            wsT_sb = consts.tile([DS, H], F32R)
            nc.sync.dma_start(wsT_sb, wsT[:, :])
            onesrow_sb = consts.tile([1, 128], F32R)
            nc.sync.dma_start(onesrow_sb, onesrow[:, :])

            # persistent tiles
            qT_sb = persist.tile([128, HC, SQ], F32R)      # Q^T/sqrt(hd): [d, q]
            semb_pc = persist.tile([128, HC], F32R)
            kbias_sb = persist.tile([128, HC], F32)
            vb_bcast = rlp.tile([128, H], F32, tag="vbb")

            # DRAM scratch
            kT_d = dram.tile([HC, 128, S], F32R)           # K^T as [dc, d_in_chunk, k]
            v_d = dram.tile([S, H], F32R)                  # V natural [k, d]
            vb_scr = dram.tile([H], F32)
            semb_scr = dram.tile([H], F32R)
            kb_scr = dram.tile([H], F32)
            l_scr = dram.tile([NH, SQ], F32)

            # ================= Stage 1: projections (eight key-blocks) ==========
            SH = S // 8      # 512 keys per block
            KH = SH // 128   # 4 key chunks per block
            with (
                tc.tile_pool(name="xtp", bufs=2) as xtp,
                tc.tile_pool(name="w1", bufs=2) as w1,
                tc.tile_pool(name="ps1", bufs=4, space="PSUM") as ps1,
                tc.tile_pool(name="pst", bufs=2, space="PSUM") as pst,
                tc.tile_pool(name="psb", bufs=2, space="PSUM") as psb,
            ):
                # --- semb = Ws @ static + bs (row layout, then roundtrip) ---
                bs_row = rlp.tile([1, H], F32, tag="row", name="bs_row")
                nc.sync.dma_start(bs_row, bsv.rearrange("d -> () d"))
                semb_row = xtp.tile([1, H], F32R, tag="srow", bufs=1)
                for d2 in range(H // 512):
                    p = psb.tile([1, 512], F32, tag="pbias", name=f"sembp{d2}")
                    nc.tensor.matmul(p[:], sd_sb[:], wsT_sb[:, d2 * 512:(d2 + 1) * 512],
                                     start=True, stop=True)
                    nc.vector.tensor_add(semb_row[:, d2 * 512:(d2 + 1) * 512], p[:],
                                         bs_row[:, d2 * 512:(d2 + 1) * 512])
                nc.sync.dma_start(semb_scr.rearrange("d -> () d"), semb_row[:])
                nc.sync.dma_start(semb_pc, semb_scr.rearrange("(c p) -> p c", p=128))

                # --- xq transpose + Q^T (scaled); wq -> wk -> wv rotate one tag ---
                wq_sb = w1.tile([128, HC, H], F32R, tag="w", name="wq")
                nc.scalar.dma_start(wq_sb, wqT.rearrange("(c p) d -> p c d", p=128))
                xqT_sb = xtp.tile([128, HC, SQ], F32R, tag="xT", name="xqT")
                for qb in range(QB):
                    xin = xtp.tile([128, H], F32R, tag="xin", bufs=3, name=f"xqin{qb}")
                    nc.sync.dma_start(xin, xq[qb * 128:(qb + 1) * 128, :].bitcast(F32R))
                    for hc in range(HC):
                        pt = pst.tile([128, 128], F32R, tag="ptr", name=f"qtr{qb}_{hc}")
                        nc.tensor.transpose(pt[:], xin[:, hc * 128:(hc + 1) * 128], ident)
                        nc.any.tensor_copy(xqT_sb[:, hc, qb * 128:(qb + 1) * 128], pt[:])
                # block-0 x transposes fill the PE while the wq DMA completes
                xT_first = xtp.tile([128, HC, SH], F32R, tag="xT", name="xT0")
                for kb in range(KH):
                    xin = xtp.tile([128, H], F32R, tag="xin", bufs=3, name=f"xin0_{kb}")
                    nc.sync.dma_start(xin, x[kb * 128:(kb + 1) * 128, :].bitcast(F32R))
                    for hc in range(HC):
                        pt = pst.tile([128, 128], F32R, tag="ptr", name=f"ptr0_{kb}_{hc}")
                        nc.tensor.transpose(pt[:], xin[:, hc * 128:(hc + 1) * 128], ident)
                        nc.any.tensor_copy(xT_first[:, hc, kb * 128:(kb + 1) * 128], pt[:])
                for dc in range(HC):
                    p = ps1.tile([128, SQ], F32, tag="pproj", name=f"qp{dc}")
                    for hc in range(HC):
                        nc.tensor.matmul(p[:], wq_sb[:, hc, dc * 128:(dc + 1) * 128],
                                         xqT_sb[:, hc, :],
                                         start=(hc == 0), stop=(hc == HC - 1))
                    nc.scalar.mul(qT_sb[:, dc, :], p[:], inv_sqrt_hd)
                if debug:
                    nc.sync.dma_start(dqT[:, :], qT_sb[:, 0, :].bitcast(F32))

                wk_sb = w1.tile([128, HC, H], F32R, tag="w", name="wk")
                nc.scalar.dma_start(wk_sb, wkT.rearrange("(c p) d -> p c d", p=128))
                wv_sb = w1.tile([128, HC, H], F32R, tag="w", name="wv")
                nc.scalar.dma_start(wv_sb, wvT.rearrange("(c p) d -> p c d", p=128))

                # --- kbias/vbias rows + roundtrips ---
                kb_row = rlp.tile([1, H], F32, tag="row", name="kb_row")
                for d2 in range(H // 512):
                    p = psb.tile([1, 512], F32, tag="pbias", name=f"kbp{d2}")
                    for hc in range(HC):
                        nc.tensor.matmul(p[:], semb_pc[:, hc:hc + 1],
                                         wk_sb[:, hc, d2 * 512:(d2 + 1) * 512],
                                         start=(hc == 0), stop=(hc == HC - 1))
                    nc.vector.tensor_copy(kb_row[:, d2 * 512:(d2 + 1) * 512], p[:])
                nc.sync.dma_start(kb_scr.rearrange("d -> () d"), kb_row[:])
                nc.sync.dma_start(kbias_sb, kb_scr.rearrange("(c p) -> p c", p=128))
                vb_row = rlp.tile([1, H], F32, tag="row", name="vb_row")
                for d2 in range(H // 512):
                    p = psb.tile([1, 512], F32, tag="pbias", name=f"vbp{d2}")
                    for hc in range(HC):
                        nc.tensor.matmul(p[:], semb_pc[:, hc:hc + 1],
                                         wv_sb[:, hc, d2 * 512:(d2 + 1) * 512],
                                         start=(hc == 0), stop=(hc == HC - 1))
                    nc.vector.tensor_copy(vb_row[:, d2 * 512:(d2 + 1) * 512], p[:])
                nc.sync.dma_start(vb_scr.rearrange("d -> () d"), vb_row[:])
                nc.sync.dma_start(vb_bcast,
                                  bass.AP(tensor=vb_scr.tensor, offset=vb_scr.offset,
                                          ap=[[0, 128], [1, H]]))

                for blk in range(8):
                    k0 = blk * KH           # first 128-chunk of this block
                    # --- transpose x rows of this block -> xT_sb [128, HC, SH] ---
                    if blk == 0:
                        xT_sb = xT_first
                    else:
                        xT_sb = xtp.tile([128, HC, SH], F32R, tag="xT", name=f"xT{blk}")
                        for kb in range(KH):
                            xin = xtp.tile([128, H], F32R, tag="xin", bufs=3,
                                           name=f"xin{blk}_{kb}")
                            nc.sync.dma_start(xin,
                                              x[(k0 + kb) * 128:(k0 + kb + 1) * 128, :]
                                              .bitcast(F32R))
                            for hc in range(HC):
                                pt = pst.tile([128, 128], F32R, tag="ptr",
                                              name=f"ptr{blk}_{kb}_{hc}")
                                nc.tensor.transpose(pt[:], xin[:, hc * 128:(hc + 1) * 128],
                                                    ident)
                                nc.any.tensor_copy(xT_sb[:, hc, kb * 128:(kb + 1) * 128],
                                                   pt[:])

                    if debug and blk == 0:
                        nc.sync.dma_start(dxT[:, :], xT_sb[:, 0, 0:512].bitcast(F32))
                    # --- K^T and V interleaved for this block ---
                    for gi in range(HC):
                        dc = gi
                        p = ps1.tile([128, 512], F32, tag="pproj", name=f"kp{blk}_{dc}")
                        for hc in range(HC):
                            nc.tensor.matmul(p[:], wk_sb[:, hc, dc * 128:(dc + 1) * 128],
                                             xT_sb[:, hc, :],
                                             start=(hc == 0), stop=(hc == HC - 1))
                        st = xtp.tile([128, 512], F32R, tag="evict", name=f"kst{blk}_{dc}")
                        nc.scalar.activation(st[:], p[:], AF.Identity,
                                             bias=kbias_sb[:, dc:dc + 1])
                        nc.sync.dma_start(kT_d[dc, :, blk * SH:(blk + 1) * SH], st[:])
                        kb, d2 = gi // 2, gi % 2
                        p = ps1.tile([128, 512], F32, tag="pproj",
                                     name=f"vp{blk}_{kb}_{d2}")
                        for hc in range(HC):
                            nc.tensor.matmul(p[:], xT_sb[:, hc, kb * 128:(kb + 1) * 128],
                                             wv_sb[:, hc, d2 * 512:(d2 + 1) * 512],
                                             start=(hc == 0), stop=(hc == HC - 1))
                        st = xtp.tile([128, 512], F32R, tag="evict",
                                      name=f"vst{blk}_{kb}_{d2}")
                        nc.vector.tensor_add(st[:], p[:],
                                             vb_bcast[:, d2 * 512:(d2 + 1) * 512])
                        nc.sync.dma_start(
                            v_d[(k0 + kb) * 128:(k0 + kb + 1) * 128,
                                d2 * 512:(d2 + 1) * 512],
                            st[:])
                if debug:
                    nc.sync.dma_start(dsemb[:, :], semb_pc[:].bitcast(F32))
                    nc.sync.dma_start(dkbias[:, :], kbias_sb[:])
                    nc.sync.dma_start(dvb[:, :], vb_bcast[0:1, :])

            # ========== Stage 2: attention per head; Stage 3: out-proj + LN =====
            with (
                tc.tile_pool(name="s2a", bufs=1) as s2a,
                tc.tile_pool(name="ps_misc", bufs=1, space="PSUM") as ps_misc,
            ):
                # prefetch out-proj weights + LN consts while attention runs
                wo_sb = s2a.tile([128, HC, H], F32R, tag="wo")
                nc.scalar.dma_start(wo_sb, woT.rearrange("(c p) d -> p c d", p=128))
                lnw_b = s2a.tile([128, H], F32, tag="lnwb")
                nc.sync.dma_start(lnw_b, bass.AP(tensor=lnw, offset=0, ap=[[0, 128], [1, H]]))
                lnb_b = s2a.tile([128, H], F32, tag="lnbb")
                nc.sync.dma_start(lnb_b, bass.AP(tensor=lnb, offset=0, ap=[[0, 128], [1, H]]))
                ctx_sb = s2a.tile([128, HC, SQ], F32R, tag="ctx")   # ctx^T/l: [d, q]

                with (
                    tc.tile_pool(name="attn", bufs=1) as attn,
                    tc.tile_pool(name="ps_s", bufs=3, space="PSUM") as ps_s,
                    tc.tile_pool(name="ps_c", bufs=1, space="PSUM") as ps_c,
                ):
                    kts = {}

                    def fetch_kt(h, kcg):
                        if (h, kcg) in kts or kcg >= KC:
                            return
                        kt = kvin.tile([128, 4, 512], F32R, tag="ktin", bufs=3,
                                       name=f"kt{h}_{kcg}")
                        nc.scalar.dma_start(
                            kt,
                            kT_d[4 * h:4 * h + 4, :, kcg * 128:kcg * 128 + 512]
                            .rearrange("c p k -> p c k"))
                        kts[(h, kcg)] = kt

                    for h in range(NH):
                        PTs = {}
                        vts = {}
                        lsum = ps_misc.tile([1, SQ], F32, tag="misc", name=f"lsum{h}")
                        ctx_ps = [ps_c.tile([128, SQ], F32, tag=f"ctx{dv}",
                                            name=f"ctxps{h}_{dv}")
                                  for dv in range(4)]

                        def emit_consume(kc, h=h, PTs=PTs, vts=vts, lsum=lsum,
                                         ctx_ps=ctx_ps):
                            PTk = PTs.pop(kc)
                            nc.tensor.matmul(lsum[:], ones_sb, PTk[:],
                                             start=(kc == 0), stop=(kc == KC - 1),
                                             skip_group_check=True)
                            vt = vts.pop(kc)
                            for dv in range(4):
                                nc.tensor.matmul(ctx_ps[dv][:],
                                                 vt[:, dv * 128:(dv + 1) * 128],
                                                 PTk[:],
                                                 start=(kc == 0), stop=(kc == KC - 1),
                                                 skip_group_check=True)

                        for kc in range(KC):
                            if kc % 4 == 0:
                                fetch_kt(h, kc)
                                # lookahead: next group, possibly next head's first
                                if kc + 4 < KC:
                                    fetch_kt(h, kc + 4)
                                elif h + 1 < NH:
                                    fetch_kt(h + 1, 0)
                            kt = kts[(h, kc // 4 * 4)]
                            # prefetch V for this chunk (consumed one iteration later)
                            vt = kvin.tile([128, HD], F32R, tag="vtin", bufs=3,
                                           name=f"vt{h}_{kc}")
                            nc.sync.dma_start(vt, v_d[kc * 128:(kc + 1) * 128,
                                                      h * HD:(h + 1) * HD])
                            vts[kc] = vt
                            ps = ps_s.tile([128, SQ], F32, tag="st", name=f"st{h}_{kc}")
                            for dq in range(4):
                                nc.tensor.matmul(
                                    ps[:],
                                    kt[:, dq, (kc % 4) * 128:(kc % 4) * 128 + 128],
                                    qT_sb[:, 4 * h + dq, :],
                                    start=(dq == 0), stop=(dq == 3))
                            PTk = attn.tile([128, SQ], F32R, tag="PTs", bufs=34,
                                            name=f"PT{h}_{kc}")
                            PTs[kc] = PTk
                            bias_ap = mb_sb[:, kc:kc + 1] if h == 0 else zb_sb
                            nc.scalar.activation(PTk[:], ps[:], AF.Exp, bias=bias_ap)
                            if debug and h == 0 and kc == 0:
                                nc.sync.dma_start(dPT[:, :], PTk[:].bitcast(F32))
                                nc.sync.dma_start(dKT[:, :, :], kt[:].bitcast(F32))
                                stdbg = rlp.tile([128, 512], F32, tag="stdbg", bufs=1)
                                nc.vector.tensor_copy(stdbg[:], ps[:])
                                nc.sync.dma_start(dST[:, :], stdbg[:])
                            if kc > 0:
                                emit_consume(kc - 1)
                            if kc == 4 * (KC // 4) - 4 and kts.get((h, kc)) is not None:
                                pass
                        emit_consume(KC - 1)
                        for key in [k for k in list(kts) if k[0] == h]:
                            kts.pop(key)
                        # evict ctx unnormalized immediately (frees PSUM banks for
                        # the next head), then normalize in place off-critical-path
                        for dv in range(4):
                            nc.scalar.copy(ctx_sb[:, 4 * h + dv, :], ctx_ps[dv][:])
                        # denominators: evict (rounds to f32r), broadcast via PE,
                        # then reciprocal across all 128 partitions at once
                        lrow = rlp.tile([1, SQ], F32R, tag="rl", name=f"lrow{h}")
                        nc.scalar.copy(lrow[:], lsum[:])
                        lb_ps = ps_s.tile([128, SQ], F32, tag="st", name=f"lbps{h}")
                        nc.tensor.matmul(lb_ps[:], onesrow_sb[:], lrow[:],
                                         start=True, stop=True)
                        rl_b = rlp.tile([128, SQ], F32, tag="rlb", name=f"rlb{h}")
                        nc.vector.reciprocal(rl_b[:], lb_ps[:])
                        if debug:
                            nc.sync.dma_start(dl[h:h + 1, :], rl_b[0:1, :])
                        for dv in range(4):
                            nc.vector.tensor_mul(ctx_sb[:, 4 * h + dv, :],
                                                 ctx_sb[:, 4 * h + dv, :], rl_b[:])

                # ---- out-proj (natural layout) + fused residual + LN ----
                if debug:
                    nc.sync.dma_start(dctx[:, :], ctx_sb[:, 0, :].bitcast(F32))
                with (
                    tc.tile_pool(name="s4", bufs=2) as s4,
                    tc.tile_pool(name="ps4", bufs=2, space="PSUM") as ps4,
                ):
                    for qb in range(QB):
                        xq_f = s4.tile([128, H], F32, tag="xqf", name=f"xqf{qb}")
                        nc.sync.dma_start(xq_f, xq[qb * 128:(qb + 1) * 128, :])
                        res_f = s4.tile([128, H], F32, tag="resf", name=f"resf{qb}")
                        for h2 in range(H // 512):
                            p = ps4.tile([128, 512], F32, tag="pout", name=f"po{qb}_{h2}")
                            for dc in range(HC):
                                nc.tensor.matmul(p[:],
                                                 ctx_sb[:, dc, qb * 128:(qb + 1) * 128],
                                                 wo_sb[:, dc, h2 * 512:(h2 + 1) * 512],
                                                 start=(dc == 0), stop=(dc == HC - 1))
                            nc.vector.tensor_add(res_f[:, h2 * 512:(h2 + 1) * 512], p[:],
                                                 xq_f[:, h2 * 512:(h2 + 1) * 512])
                        if debug and qb == 0:
                            nc.sync.dma_start(doutT[:, :], res_f[:, 0:512])
                        # LayerNorm via bn_stats; small tiles packed into one slot
                        # cols: 0:12 stats | 12:14 mv | 14 sd | 15 rstd
                        LS = s4.tile([128, 16], F32, tag="lns", name=f"lns{qb}")
                        for h2 in range(H // 512):
                            nc.vector.bn_stats(
                                LS[:, h2 * 6:(h2 + 1) * 6]
                                .rearrange("p (a b) -> p a b", a=1),
                                res_f[:, h2 * 512:(h2 + 1) * 512])
                        nc.vector.bn_aggr(LS[:, 12:14], LS[:, 0:12]
                                          .rearrange("p (a b) -> p a b", a=2))
                        nc.scalar.activation(LS[:, 14:15], LS[:, 13:14], AF.Sqrt,
                                             bias=eps_sb)
                        nc.vector.reciprocal(LS[:, 15:16], LS[:, 14:15])
                        norm = s4.tile([128, H], F32, tag="norm", name=f"norm{qb}", bufs=1)
                        scl = s4.tile([128, H], F32, tag="scl", name=f"scl{qb}", bufs=1)
                        fin = s4.tile([128, H], F32, tag="fin", name=f"fin{qb}")
                        for h2 in range(H // 512):
                            sl = slice(h2 * 512, (h2 + 1) * 512)
                            nc.vector.tensor_scalar(norm[:, sl], res_f[:, sl],
                                                    LS[:, 12:13], LS[:, 15:16],
                                                    ALU.subtract, ALU.mult)
                            nc.vector.tensor_mul(scl[:, sl], norm[:, sl], lnw_b[:, sl])
                            nc.vector.tensor_add(fin[:, sl], scl[:, sl], lnb_b[:, sl])
                            nc.sync.dma_start(out[qb * 128:(qb + 1) * 128, sl],
                                              fin[:, sl])

    nc.compile()
    return nc


_CACHED_NC = {}


def _get_nc(debug=False):
    if debug not in _CACHED_NC:
        _CACHED_NC[debug] = build_program(debug)
    return _CACHED_NC[debug]


def _prep_inputs(inputs, static_data, base_mask, Wq, Wk, Wv, Wo, Ws, bs, ln_w, ln_b):
    f32 = np.float32
    xf = np.ascontiguousarray(inputs, f32)
    common = {
        "x": xf,
        "wqT": np.ascontiguousarray(np.asarray(Wq, f32).T),
        "wkT": np.ascontiguousarray(np.asarray(Wk, f32).T),
        "wvT": np.ascontiguousarray(np.asarray(Wv, f32).T),
        "woT": np.ascontiguousarray(np.asarray(Wo, f32).T),
        "wsT": np.ascontiguousarray(np.asarray(Ws, f32).T),
        "sdat": np.ascontiguousarray(np.asarray(static_data, f32).reshape(DS, 1)),
        "bsv": np.ascontiguousarray(bs, f32),
        "mbias": np.ascontiguousarray(np.where(np.asarray(base_mask, bool), 0.0, -1e30).astype(f32).reshape(KC, 128).T),
        "onescol": np.ones((128, 1), f32),
        "onesrow": np.ones((1, 128), f32),
        "identd": np.eye(128, dtype=f32),
        "lnw": np.ascontiguousarray(ln_w, f32),
        "lnb": np.ascontiguousarray(ln_b, f32),
    }
    in_maps = []
    for c in range(NC):
        m = dict(common)
        m["xq"] = np.ascontiguousarray(xf[c * SQ:(c + 1) * SQ, :])
        in_maps.append(m)
    return in_maps


def kernel_run(trace=False, debug=False, **inputs):
    nc = _get_nc(debug)
    in_maps = _prep_inputs(**inputs)
    res = run_bass_kernel_spmd(nc, in_maps, core_ids=list(range(NC)), trace=trace)
    outp = np.concatenate([res.results[c]["out"] for c in range(NC)], axis=0)
    return outp, res


def kernel(**inputs):
    outp, _ = kernel_run(trace=False, **inputs)
    return outp



# revision 13
# speedup vs baseline: 1.2959x; 1.2959x over previous
"""Trainium2 Bass kernel for nn_AttentionBlock (S=4096, H=1024, NH=2, DS=64).

Strategy v2: sequence parallelism over queries (512/core on 8 cores) plus
pair-split K/V projections. Cores 2k and 2k+1 share an HBM region
(addr_space="Shared" is pair-shared on TRN2); each core computes K^T/V for 4
of the 8 key blocks (parity-interleaved), writes them to the pair-shared
region (one If on a parity register), a tiny AllGather acts as the pair
barrier, then every core reads all 8 block slots back. All matmul operands
are bf16 (full PE rate, SBUF/DMA halved); accumulation stays f32 in PSUM.
semb/kbias/vbias are folded on the host.

Per core:
  stage1: transpose own 4 blocks (f32r PE transpose -> bf16), K^T/V
          projections (bias fused into eviction) -> shared writes ->
          barrier; Q^T (1/sqrt(hd) folded) overlaps the writes/barrier.
  stage2: per head, per slot: S^T chunks -> fused exp+mask (ACT) -> lsum via
          ones-matmul -> ctx^T accumulated in PSUM; normalized by 1/lsum on
          eviction. A tail AllGather keeps the next iteration's writes from
          racing this iteration's reads.
  stage3: out-projection in natural [q, H] layout, fused residual,
          LayerNorm via bn_stats.
"""

import math
import sys

sys.path.insert(0, "/opt/trn_rl_repo")

import numpy as np
import ml_dtypes

import concourse.bass as bass
import concourse.mybir as mybir
import concourse.tile as tile
from concourse import bacc
from concourse.bass_utils import run_bass_kernel_spmd
from concourse.tile_rust import add_dep_helper

S, H, NH, DS = 4096, 1024, 2, 64
HD = H // NH            # 512
NC = 8                  # cores
SQ = S // NC            # 512 queries per core
NB = 8                  # key blocks of 512
EPS = 1e-5
F32 = mybir.dt.float32
F32R = mybir.dt.float32r
BF16 = mybir.dt.bfloat16
I32 = mybir.dt.int32
U8 = mybir.dt.uint8
AF = mybir.ActivationFunctionType
ALU = mybir.AluOpType

HC = H // 128           # 8 hidden chunks
QB = SQ // 128          # 4 query chunks of 128
KC = S // 128           # 32 key chunks of 128
SLOT = 8192             # bf16 elems per block slot: [K_h0 2048|V_h0 2048|K_h1|V_h1]

inv_sqrt_hd = 1.0 / math.sqrt(HD)


def my_blocks(c):
    """Blocks core c computes: own query block first, then same-parity rest."""
    rest = [b for b in range(c % 2, NB, 2) if b != c]
    return [c] + rest


def build_program():
    nc = bacc.Bacc("TRN2", target_bir_lowering=False, debug=False, num_devices=NC)

    # ---- DRAM I/O (per-core inputs prepared on host) ----
    xkv = nc.dram_tensor("xkv", [4 * 512, H], F32, kind="ExternalInput")
    wqT = nc.dram_tensor("wqT", [H, H], BF16, kind="ExternalInput")
    wkT = nc.dram_tensor("wkT", [H, H], BF16, kind="ExternalInput")
    wvT = nc.dram_tensor("wvT", [H, H], BF16, kind="ExternalInput")
    woT = nc.dram_tensor("woT", [H, H], BF16, kind="ExternalInput")
    kbias = nc.dram_tensor("kbias", [128, HC], F32, kind="ExternalInput")
    vbias = nc.dram_tensor("vbias", [H], F32, kind="ExternalInput")
    mbias = nc.dram_tensor("mbias", [128, KC], F32, kind="ExternalInput")
    identd = nc.dram_tensor("identd", [128, 128], F32R, kind="ExternalInput")
    onescol = nc.dram_tensor("onescol", [128, 1], BF16, kind="ExternalInput")
    onesrow = nc.dram_tensor("onesrow", [1, 128], F32R, kind="ExternalInput")
    lnw = nc.dram_tensor("lnw", [H], F32, kind="ExternalInput")
    lnb = nc.dram_tensor("lnb", [H], F32, kind="ExternalInput")
    parity = nc.dram_tensor("parity", [1, 1], I32, kind="ExternalInput")
    out = nc.dram_tensor("out", [SQ, H], F32, kind="ExternalOutput")

    # pair-shared exchange region + barrier bounce buffers
    kv_shared = nc.dram_tensor("kv_shared", [NB, 128, SLOT], BF16, addr_space="Shared")
    bar_in1 = nc.dram_tensor("bar_in1", [1, 1], U8)
    bar_out1 = nc.dram_tensor("bar_out1", [NC, 1], U8)
    bar_in2 = nc.dram_tensor("bar_in2", [1, 1], U8)
    bar_out2 = nc.dram_tensor("bar_out2", [NC, 1], U8)

    with tile.TileContext(nc) as tc:
        with (
            tc.tile_pool(name="consts", bufs=1) as consts,
            tc.tile_pool(name="persist", bufs=1) as persist,
        ):
            ident = consts.tile([128, 128], F32R)
            nc.sync.dma_start(ident, identd[:, :])
            ones_bf = consts.tile([128, 1], BF16)
            nc.sync.dma_start(ones_bf, onescol[:, :])
            onesrow_fr = consts.tile([1, 128], F32R)
            nc.sync.dma_start(onesrow_fr, onesrow[:, :])
            Af = consts.tile([128, 36], F32)   # 0:32 maskbias | 32 zero | 33 eps
            mb_sb = Af[:, 0:32]
            nc.sync.dma_start(mb_sb, mbias[:, :])
            zb_sb = Af[:, 32:33]
            nc.vector.memset(zb_sb, 0.0)
            eps_sb = Af[:, 33:34]
            nc.vector.memset(eps_sb, EPS)
            kb_sb = consts.tile([128, HC], F32)
            nc.sync.dma_start(kb_sb, kbias[:, :])
            vb_bcast = consts.tile([128, H], F32)
            nc.sync.dma_start(vb_bcast,
                              bass.AP(tensor=vbias, offset=0, ap=[[0, 128], [1, H]]))

            # parity register on sync engine
            idt = consts.tile([1, 1], I32)
            nc.sync.dma_start(idt[:], parity[:, :])
            preg = nc.sync.alloc_register("parity_reg")
            nc.sync.load(preg, idt[0:1, 0:1])

            qT = persist.tile([128, HC, SQ], BF16)

            # ================= Stage 1: projections for my 4 blocks ==========
            with (
                tc.tile_pool(name="kvop", bufs=1) as kvop,
                tc.tile_pool(name="wqp", bufs=1) as wqp,
                tc.tile_pool(name="w1", bufs=2) as w1,
                tc.tile_pool(name="xt0p", bufs=1) as xt0p,
                tc.tile_pool(name="xtp", bufs=2) as xtp,
                tc.tile_pool(name="ps1", bufs=3, space="PSUM") as ps1,
                tc.tile_pool(name="pst", bufs=2, space="PSUM") as pst,
            ):
                kv_own = kvop.tile([128, 4, SLOT], BF16)
                wk_sb = w1.tile([128, HC, H], BF16, tag="w", name="wk")
                nc.scalar.dma_start(wk_sb, wkT.rearrange("(c p) d -> p c d", p=128))
                wq_sb = wqp.tile([128, HC, H], BF16, name="wq")
                nc.scalar.dma_start(wq_sb, wqT.rearrange("(c p) d -> p c d", p=128))
                wv_sb = w1.tile([128, HC, H], BF16, tag="w", name="wv")
                nc.scalar.dma_start(wv_sb, wvT.rearrange("(c p) d -> p c d", p=128))

                xT0 = xt0p.tile([128, HC, 512], BF16, name="xT0")
                for j in range(4):
                    # transpose block j rows -> xT_j [128, hc, 512] bf16
                    if j == 0:
                        xT_j = xT0
                    else:
                        xT_j = xtp.tile([128, HC, 512], BF16, tag="xT", name=f"xT{j}")
                    for kb in range(4):
                        xin = xtp.tile([128, H], F32R, tag="xin", bufs=3,
                                       name=f"xin{j}_{kb}")
                        nc.sync.dma_start(
                            xin, xkv[(4 * j + kb) * 128:(4 * j + kb + 1) * 128, :]
                            .bitcast(F32R))
                        for hc in range(HC):
                            pt = pst.tile([128, 128], F32R, tag="ptr",
                                          name=f"ptr{j}_{kb}_{hc}")
                            nc.tensor.transpose(pt[:], xin[:, hc * 128:(hc + 1) * 128],
                                                ident)
                            nc.any.tensor_copy(xT_j[:, hc, kb * 128:(kb + 1) * 128],
                                               pt[:])
                    # K^T projection for block j (both heads)
                    for h in range(NH):
                        for dq in range(4):
                            dc = 4 * h + dq
                            p = ps1.tile([128, 512], F32, tag="pp", name=f"kp{j}_{dc}")
                            for hc in range(HC):
                                nc.tensor.matmul(p[:],
                                                 wk_sb[:, hc, dc * 128:(dc + 1) * 128],
                                                 xT_j[:, hc, :],
                                                 start=(hc == 0), stop=(hc == HC - 1))
                            nc.scalar.activation(
                                kv_own[:, j, h * 4096 + dq * 512:h * 4096 + (dq + 1) * 512],
                                p[:], AF.Identity, bias=kb_sb[:, dc:dc + 1])
                    # V projection for block j (natural layout)
                    for kb in range(4):
                        for h in range(NH):
                            p = ps1.tile([128, 512], F32, tag="pp",
                                         name=f"vp{j}_{kb}_{h}")
                            for hc in range(HC):
                                nc.tensor.matmul(p[:],
                                                 xT_j[:, hc, kb * 128:(kb + 1) * 128],
                                                 wv_sb[:, hc, h * 512:(h + 1) * 512],
                                                 start=(hc == 0), stop=(hc == HC - 1))
                            nc.vector.tensor_add(
                                kv_own[:, j,
                                       h * 4096 + 2048 + kb * 512:h * 4096 + 2048 + (kb + 1) * 512],
                                p[:], vb_bcast[:, h * 512:(h + 1) * 512])

                # ---- write own half to pair-shared DRAM (parity slots) ----
                wr_insts = []
                with tc.If(nc.sync.snap(preg) == 0) as cmp:
                    for j in range(4):
                        w_ = nc.sync.dma_start(kv_shared[j, :, :], kv_own[:, j, :])
                        wr_insts.append(w_)
                with cmp.Else():
                    for j in range(4):
                        w_ = nc.sync.dma_start(kv_shared[4 + j, :, :], kv_own[:, j, :])
                        wr_insts.append(w_)

                # ---- pair barrier (writes visible to partner after this) ----
                cc1 = nc.gpsimd.collective_compute(
                    "AllGather", ALU.bypass, replica_groups=[list(range(NC))],
                    ins=[bar_in1[:].opt()], outs=[bar_out1[:].opt()])
                for w_ in wr_insts:
                    add_dep_helper(cc1.ins, w_.ins, reason="barrier after kv writes")

                # ---- Q^T (overlaps writes/barrier on PE), scaled ----
                for dc in range(HC):
                    p = ps1.tile([128, SQ], F32, tag="pp", name=f"qp{dc}")
                    for hc in range(HC):
                        nc.tensor.matmul(p[:],
                                         wq_sb[:, hc, dc * 128:(dc + 1) * 128],
                                         xT0[:, hc, :],
                                         start=(hc == 0), stop=(hc == HC - 1))
                    nc.scalar.mul(qT[:, dc, :], p[:], inv_sqrt_hd)

            # ========== Stage 2: attention; Stage 3: out-proj + LN =====
            with (
                tc.tile_pool(name="s2a", bufs=1) as s2a,
                tc.tile_pool(name="ps_misc", bufs=1, space="PSUM") as ps_misc,
            ):
                # prefetch out-proj weights + LN consts while attention runs
                wo_sb = s2a.tile([128, HC, H], BF16, tag="wo")
                nc.scalar.dma_start(wo_sb, woT.rearrange("(c p) d -> p c d", p=128))
                lnw_b = s2a.tile([128, H], F32, tag="lnwb")
                nc.sync.dma_start(lnw_b, bass.AP(tensor=lnw, offset=0, ap=[[0, 128], [1, H]]))
                lnb_b = s2a.tile([128, H], F32, tag="lnbb")
                nc.sync.dma_start(lnb_b, bass.AP(tensor=lnb, offset=0, ap=[[0, 128], [1, H]]))
                ctx_sb = s2a.tile([128, HC, SQ], BF16, tag="ctx")   # ctx^T/l: [d, q]

                rd_insts = []
                with (
                    tc.tile_pool(name="kvp", bufs=1) as kvp,
                    tc.tile_pool(name="attn", bufs=1) as attn,
                    tc.tile_pool(name="rlp", bufs=1) as rlp,
                    tc.tile_pool(name="ps_s", bufs=3, space="PSUM") as ps_s,
                    tc.tile_pool(name="ps_c", bufs=1, space="PSUM") as ps_c,
                ):
                    kv_all = kvp.tile([128, NB, SLOT], BF16, tag="kva")
                    for s in range(NB):
                        r_ = nc.sync.dma_start(kv_all[:, s, :], kv_shared[s, :, :])
                        add_dep_helper(r_.ins, cc1.ins, reason="read after barrier")
                        rd_insts.append(r_)

                    for h in range(NH):
                        lsum = ps_misc.tile([1, SQ], F32, tag="misc", name=f"lsum{h}")
                        ctx_ps = [ps_c.tile([128, SQ], F32, tag=f"ctx{dv}",
                                            name=f"ctxps{h}_{dv}")
                                  for dv in range(4)]
                        for s in range(NB):
                            for j in range(4):
                                kc = s * 4 + j
                                ps = ps_s.tile([128, SQ], F32, tag="st",
                                               name=f"st{h}_{kc}")
                                for dq in range(4):
                                    o0 = h * 4096 + dq * 512 + j * 128
                                    nc.tensor.matmul(
                                        ps[:], kv_all[:, s, o0:o0 + 128],
                                        qT[:, 4 * h + dq, :],
                                        start=(dq == 0), stop=(dq == 3))
                                PTk = attn.tile([128, SQ], BF16, tag="PTs", bufs=6,
                                                name=f"PT{h}_{kc}")
                                bias_ap = mb_sb[:, kc:kc + 1] if h == 0 else zb_sb
                                nc.scalar.activation(PTk[:], ps[:], AF.Exp, bias=bias_ap)
                                nc.tensor.matmul(lsum[:], ones_bf, PTk[:],
                                                 start=(kc == 0), stop=(kc == KC - 1),
                                                 skip_group_check=True)
                                for dv in range(4):
                                    o1 = h * 4096 + 2048 + j * 512 + dv * 128
                                    nc.tensor.matmul(
                                        ctx_ps[dv][:], kv_all[:, s, o1:o1 + 128],
                                        PTk[:],
                                        start=(kc == 0), stop=(kc == KC - 1),
                                        skip_group_check=True)
                        # denominators -> broadcast -> fused normalize on eviction
                        lrow = rlp.tile([1, SQ], F32R, tag="rl", name=f"lrow{h}")
                        nc.scalar.copy(lrow[:], lsum[:])
                        lb_ps = ps_s.tile([128, SQ], F32, tag="st", name=f"lbps{h}")
                        nc.tensor.matmul(lb_ps[:], onesrow_fr[:], lrow[:],
                                         start=True, stop=True)
                        rl_b = rlp.tile([128, SQ], F32, tag="rlb", name=f"rlb{h}")
                        nc.vector.reciprocal(rl_b[:], lb_ps[:])
                        for dv in range(4):
                            nc.vector.tensor_mul(ctx_sb[:, 4 * h + dv, :],
                                                 ctx_ps[dv][:], rl_b[:])

                # ---- tail barrier: partner done reading before next iter ----
                cc2 = nc.gpsimd.collective_compute(
                    "AllGather", ALU.bypass, replica_groups=[list(range(NC))],
                    ins=[bar_in2[:].opt()], outs=[bar_out2[:].opt()])
                for r_ in rd_insts:
                    add_dep_helper(cc2.ins, r_.ins, reason="tail barrier after reads")

                # ---- out-proj (natural layout) + fused residual + LN ----
                with (
                    tc.tile_pool(name="s4", bufs=2) as s4,
                    tc.tile_pool(name="ps4", bufs=2, space="PSUM") as ps4,
                ):
                    for qb in range(QB):
                        xq_f = s4.tile([128, H], F32, tag="xqf", name=f"xqf{qb}")
                        nc.sync.dma_start(xq_f, xkv[qb * 128:(qb + 1) * 128, :])
                        res_f = s4.tile([128, H], F32, tag="resf", name=f"resf{qb}")
                        for h2 in range(H // 512):
                            p = ps4.tile([128, 512], F32, tag="pout", name=f"po{qb}_{h2}")
                            for dc in range(HC):
                                nc.tensor.matmul(p[:],
                                                 ctx_sb[:, dc, qb * 128:(qb + 1) * 128],
                                                 wo_sb[:, dc, h2 * 512:(h2 + 1) * 512],
                                                 start=(dc == 0), stop=(dc == HC - 1))
                            nc.vector.tensor_add(res_f[:, h2 * 512:(h2 + 1) * 512], p[:],
                                                 xq_f[:, h2 * 512:(h2 + 1) * 512])
                        # LayerNorm via bn_stats; small tiles packed into one slot
                        LS = s4.tile([128, 16], F32, tag="lns", name=f"lns{qb}")
                        for h2 in range(H // 512):
                            nc.vector.bn_stats(
                                LS[:, h2 * 6:(h2 + 1) * 6]
                                .rearrange("p (a b) -> p a b", a=1),
                                res_f[:, h2 * 512:(h2 + 1) * 512])
                        nc.vector.bn_aggr(LS[:, 12:14], LS[:, 0:12]
                                          .rearrange("p (a b) -> p a b", a=2))
                        nc.scalar.activation(LS[:, 14:15], LS[:, 13:14], AF.Sqrt,
                                             bias=eps_sb)
                        nc.vector.reciprocal(LS[:, 15:16], LS[:, 14:15])
                        norm = s4.tile([128, H], F32, tag="norm", name=f"norm{qb}", bufs=1)
                        scl = s4.tile([128, H], F32, tag="scl", name=f"scl{qb}", bufs=1)
                        fin = s4.tile([128, H], F32, tag="fin", name=f"fin{qb}")
                        for h2 in range(H // 512):
                            sl = slice(h2 * 512, (h2 + 1) * 512)
                            nc.vector.tensor_scalar(norm[:, sl], res_f[:, sl],
                                                    LS[:, 12:13], LS[:, 15:16],
                                                    ALU.subtract, ALU.mult)
                            nc.vector.tensor_mul(scl[:, sl], norm[:, sl], lnw_b[:, sl])
                            nc.vector.tensor_add(fin[:, sl], scl[:, sl], lnb_b[:, sl])
                            nc.sync.dma_start(out[qb * 128:(qb + 1) * 128, sl],
                                              fin[:, sl])

    nc.compile()
    return nc


_CACHED_NC = {}


def _get_nc():
    if "nc" not in _CACHED_NC:
        _CACHED_NC["nc"] = build_program()
    return _CACHED_NC["nc"]


def _prep_inputs(inputs, static_data, base_mask, Wq, Wk, Wv, Wo, Ws, bs, ln_w, ln_b):
    f32 = np.float32
    bf16 = ml_dtypes.bfloat16
    xf = np.ascontiguousarray(inputs, f32)
    Wk_f = np.asarray(Wk, f32)
    Wv_f = np.asarray(Wv, f32)
    semb = np.asarray(Ws, f32) @ np.asarray(static_data, f32) + np.asarray(bs, f32)
    kbias_full = Wk_f @ semb                     # [H]
    vbias_full = Wv_f @ semb                     # [H]
    maskbias = np.where(np.asarray(base_mask, bool), 0.0, -1e30).astype(f32)  # [S]

    common = {
        "wqT": np.ascontiguousarray(np.asarray(Wq, f32).T).astype(bf16),
        "wkT": np.ascontiguousarray(Wk_f.T).astype(bf16),
        "wvT": np.ascontiguousarray(Wv_f.T).astype(bf16),
        "woT": np.ascontiguousarray(np.asarray(Wo, f32).T).astype(bf16),
        "kbias": np.ascontiguousarray(kbias_full.reshape(HC, 128).T),
        "vbias": np.ascontiguousarray(vbias_full),
        "identd": np.eye(128, dtype=f32),
        "onescol": np.ones((128, 1), bf16),
        "onesrow": np.ones((1, 128), f32),
        "lnw": np.ascontiguousarray(ln_w, f32),
        "lnb": np.ascontiguousarray(ln_b, f32),
    }
    in_maps = []
    for c in range(NC):
        m = dict(common)
        mine = my_blocks(c)
        # kv_shared slots: [even core's 4 blocks | odd core's 4 blocks]
        order = my_blocks(c & ~1) + my_blocks(c | 1)
        m["xkv"] = np.ascontiguousarray(
            np.concatenate([xf[b * 512:(b + 1) * 512, :] for b in mine], axis=0))
        mb = np.empty((128, KC), f32)
        for s8 in range(NB):
            b = order[s8]
            for j in range(4):
                mb[:, s8 * 4 + j] = maskbias[(b * 4 + j) * 128:(b * 4 + j + 1) * 128]
        m["mbias"] = mb
        m["parity"] = np.array([[c % 2]], np.int32)
        in_maps.append(m)
    return in_maps


def kernel_run(trace=False, **inputs):
    nc = _get_nc()
    in_maps = _prep_inputs(**inputs)
    res = run_bass_kernel_spmd(nc, in_maps, core_ids=list(range(NC)), trace=trace)
    outp = np.concatenate([np.asarray(res.results[c]["out"], np.float32)
                           for c in range(NC)], axis=0)
    return outp, res


def kernel(**inputs):
    outp, _ = kernel_run(trace=False, **inputs)
    return outp
